# revision 1
# baseline (speedup 1.0000x reference)
"""Trainium2 Bass kernel for a dense transformer block (LN->causal attn->res->LN->MLP->res).

Shapes (hardcoded): x [2, 2048, 1024], 16 heads, head_dim 64, MLP hidden 4096, fp32 in/out.

Sharding: 8 cores = (batch b in {0,1}) x (sequence chunk j in {0..3}, 512 tokens).
Each core receives its batch's full 2048-token context, ROTATED so that its own
chunk sits in the last 512 positions.  Causality is enforced by per-core DATA:
a per-key-tile additive bias (0 past / -30000 future) folded into the exp on
ACT, plus a triangular 0/1 mask multiplied onto the diagonal (last 4) key
tiles.  Each core computes LN1 + K/V over the whole context, Q/attention/LN2/
MLP for its own 512 tokens, and writes its [512, 1024] output slice.  No
cross-core communication.

v2 notes (vs the earlier DRAM-bounce version):
  * All matmul operands are bf16 (PSUM accumulation fp32).  K^T and Q^T stay
    SBUF-resident in bf16 -- no DRAM round trip for K.
  * All weights arrive host-pre-tiled so every weight DMA reads >=2KB
    contiguous runs per partition (the DMA descriptor model halves throughput
    below 512B runs).
  * P1/P4 transposes use the DMA xbar transpose (16x128 tiles, bf16) straight
    into SBUF -- no PE transpose + PSUM bounce for LN outputs.
  * wfc (all 32 tiles, bf16) is prefetched during attention, so the MLP fc
    phase runs without DMA stalls; wproj streams double-buffered.
  * V is augmented with a per-head all-ones 65th column accumulating the
    softmax denominator (normalized after AV).
"""

from contextlib import ExitStack

import numpy as np

import concourse.bacc as bacc
import concourse.mybir as mybir
import concourse.tile as tile
from concourse.masks import make_identity

F32 = mybir.dt.float32
BF16 = mybir.dt.bfloat16
AF = mybir.ActivationFunctionType
ALU = mybir.AluOpType

B = 2
T = 2048
D = 1024
H = 16
HD = 64
HDA = HD + 1  # +1 denominator column per head
MLP = 4096
NQ = 512  # tokens per core
CTX = T
EPS = 1e-5
NEG = -30000.0

N_CORES = 8
P = 128

KT_T = CTX // P  # 16 key tiles
D_T = D // P  # 8
Q_T = NQ // P  # 4
M_T = MLP // P  # 32
VA = H * HDA  # 1040 augmented V width


def build_program(loop_n: int = 1, bv_nonzero: bool = False):
    """Emit the SPMD Bass program. Returns finalized nc."""
    nc = bacc.Bacc("TRN2", target_bir_lowering=False)

    xc = nc.dram_tensor("xc", [CTX, D], F32, kind="ExternalInput")
    xcb = nc.dram_tensor("xcb", [CTX, D], BF16, kind="ExternalInput")
    wqp = nc.dram_tensor("wqp", [D, D], BF16, kind="ExternalInput")
    wkp = nc.dram_tensor("wkp", [D, D], BF16, kind="ExternalInput")
    bqk = nc.dram_tensor("bqk", [P, 2 * D_T], F32, kind="ExternalInput")
    wva = nc.dram_tensor("wva", [D, VA], BF16, kind="ExternalInput")
    bva = nc.dram_tensor("bva", [1, VA], BF16, kind="ExternalInput")
    biask = nc.dram_tensor("biask", [P, KT_T], F32, kind="ExternalInput")
    trimask = nc.dram_tensor("trimask", [P, 4 * 2 * NQ], BF16, kind="ExternalInput")
    wfcp = nc.dram_tensor("wfcp", [MLP, D], BF16, kind="ExternalInput")
    bfc = nc.dram_tensor("bfc", [P, M_T], F32, kind="ExternalInput")
    wprojp = nc.dram_tensor("wprojp", [D, MLP], BF16, kind="ExternalInput")
    bproj = nc.dram_tensor("bproj", [P, D_T], F32, kind="ExternalInput")
    out = nc.dram_tensor("out", [NQ, D], F32, kind="ExternalOutput")

    with tile.TileContext(nc) as tc:
        with ExitStack() as ctx:
            if loop_n > 1:
                ctx.enter_context(tc.For_i(0, loop_n, 1))
            const = ctx.enter_context(tc.tile_pool(name="const", bufs=1))
            identity = const.tile([P, P], F32)
            make_identity(nc, identity)
            identity_bf = const.tile([P, P], BF16)
            make_identity(nc, identity_bf)
            ones1 = const.tile([1, P], BF16)
            nc.vector.memset(ones1, 1.0)
            eps_t = const.tile([P, 1], F32)
            nc.vector.memset(eps_t, EPS)
            bqk_sb = const.tile([P, 2 * D_T], F32)
            nc.sync.dma_start(bqk_sb, bqk[:, :])
            bva_sb = const.tile([1, VA], BF16)
            nc.sync.dma_start(bva_sb, bva[:, :])
            biask_sb = const.tile([P, KT_T], F32)
            nc.sync.dma_start(biask_sb, biask[:, :])

            # Long-lived pools (closed explicitly at phase boundaries).
            qt_cm = tc.tile_pool(name="qt", bufs=1)
            qt_pool = qt_cm.__enter__()
            QT = [qt_pool.tile([P, NQ], BF16, name=f"QT{i}") for i in range(D_T)]
            kt_cm = tc.tile_pool(name="ktp", bufs=1)
            kt_pool = kt_cm.__enter__()
            KT = [kt_pool.tile([P, CTX], BF16, name=f"KT{i}") for i in range(D_T)]
            vsb_cm = tc.tile_pool(name="vsb", bufs=1)
            vsb_pool = vsb_cm.__enter__()
            VSB = [vsb_pool.tile([P, VA], BF16, name=f"VSB{i}") for i in range(KT_T)]

            # RIGHT pools (live into P4/P5)
            yt_pool = ctx.enter_context(tc.tile_pool(name="yt", bufs=1, side="right"))
            YT = [yt_pool.tile([P, NQ], F32, name=f"YT{i}") for i in range(D_T)]
            x2_pool = ctx.enter_context(tc.tile_pool(name="x2", bufs=1, side="right"))
            X2 = [x2_pool.tile([P, D], F32, name=f"X2{i}") for i in range(Q_T)]
            l2t_pool = ctx.enter_context(
                tc.tile_pool(name="l2t", bufs=1, side="right")
            )
            L2T = [l2t_pool.tile([P, NQ], BF16, name=f"L2T{i}") for i in range(D_T)]
            # xnt sits on TOP of the RIGHT stack so it can be closed right
            # after V production (the left stack still has attention pools
            # open); wfc prefetch then reuses its space.
            xnt_cm = tc.tile_pool(name="xnt", bufs=1, side="right")
            xnt_pool = xnt_cm.__enter__()
            xnT = [xnt_pool.tile([P, CTX], BF16, name=f"xnT{i}") for i in range(D_T)]

            # ---------------- P1: load x, LN1, PE-transpose -> xnT (bf16) ----
            # Own-chunk tiles (tt 12..15) first so Q production can start
            # early; x loaded in 4-tile batches (one DMA each, ~625ns HWDGE
            # per DMA instruction makes small DMAs expensive).
            with tc.tile_pool(name="p1work", bufs=2) as p1w, tc.tile_pool(
                name="p1xn", bufs=4
            ) as p1xn, tc.tile_pool(name="p1stat", bufs=6) as p1s, tc.tile_pool(
                name="p1ps", bufs=6, space="PSUM"
            ) as p1ps:
                for bt in ((12, 13, 14, 15), (0, 1, 2, 3), (4, 5, 6, 7), (8, 9, 10, 11)):
                    xt = p1w.tile([P, 4, D], BF16, tag="xt")
                    nc.sync.dma_start(
                        xt,
                        xcb[bt[0] * P : (bt[0] + 4) * P, :].rearrange(
                            "(a p) c -> p a c", p=P
                        ),
                    )
                    xns = []
                    for ai, tt in enumerate(bt):
                        stats = p1s.tile([P, 2, 6], F32, tag="stats")
                        for g in range(2):
                            nc.vector.bn_stats(
                                stats[:, g, :], xt[:, ai, g * 512 : (g + 1) * 512]
                            )
                        mv = p1s.tile([P, 2], F32, tag="mv")
                        nc.vector.bn_aggr(mv, stats)
                        sd = p1s.tile([P, 1], F32, tag="sd")
                        nc.scalar.activation(sd, mv[:, 1:2], AF.Sqrt, bias=eps_t)
                        rstd = p1s.tile([P, 1], F32, tag="rstd")
                        nc.vector.reciprocal(rstd, sd)
                        nmb = p1s.tile([P, 1], F32, tag="nmb")
                        nc.vector.tensor_scalar(
                            nmb, mv[:, 0:1], rstd, -1.0, ALU.mult, ALU.mult
                        )
                        xn = p1xn.tile([P, D], BF16, tag="xn")
                        nc.scalar.activation(
                            xn, xt[:, ai, :], AF.Identity, bias=nmb, scale=rstd
                        )
                        xns.append(xn)
                    # 4 transposes per dt into one PSUM tile, then a single
                    # 512-wide copy (amortizes the per-instruction access
                    # overhead on DVE/ACT).
                    for dt_ in range(D_T):
                        tp = p1ps.tile([P, 4, P], BF16, tag="tp")
                        for ai in range(4):
                            nc.tensor.transpose(
                                tp[:, ai, :],
                                xns[ai][:, dt_ * P : (dt_ + 1) * P],
                                identity_bf,
                            )
                        if dt_ % 2 == 0:
                            nc.vector.tensor_copy(
                                xnT[dt_][:, bt[0] * P : (bt[0] + 4) * P], tp
                            )
                        else:
                            nc.scalar.copy(
                                xnT[dt_][:, bt[0] * P : (bt[0] + 4) * P], tp
                            )

            # ---------------- P2a: Q^T (bf16, own 512 tokens) ----------------
            with tc.tile_pool(name="p2q", bufs=2) as p2q, tc.tile_pool(
                name="p2qps", bufs=3, space="PSUM"
            ) as p2qps:
                for mt in range(D_T):
                    ws = p2q.tile([P, D_T, P], BF16, tag="wsq")
                    nc.sync.dma_start(ws, wqp[mt * P : (mt + 1) * P, :])
                    ps = p2qps.tile([P, NQ], F32, tag="ps")
                    for kt_ in range(D_T):
                        nc.tensor.matmul(
                            ps,
                            ws[:, kt_, :],
                            xnT[kt_][:, CTX - NQ :],
                            start=(kt_ == 0),
                            stop=(kt_ == D_T - 1),
                        )
                    nc.scalar.activation(
                        QT[mt], ps, AF.Identity, bias=bqk_sb[:, mt : mt + 1]
                    )

            # ---------------- P2b: K^T (bf16, SBUF resident, full ctx) -------
            with tc.tile_pool(name="p2k", bufs=2) as p2k, tc.tile_pool(
                name="p2kps", bufs=3, space="PSUM"
            ) as p2kps:
                for mt in range(D_T):
                    ws = p2k.tile([P, D_T, P], BF16, tag="wsk")
                    nc.sync.dma_start(ws, wkp[mt * P : (mt + 1) * P, :])
                    for nt in range(CTX // 512):
                        ps = p2kps.tile([P, 512], F32, tag="ps")
                        for kt_ in range(D_T):
                            nc.tensor.matmul(
                                ps,
                                ws[:, kt_, :],
                                xnT[kt_][:, nt * 512 : (nt + 1) * 512],
                                start=(kt_ == 0),
                                stop=(kt_ == D_T - 1),
                            )
                        nc.vector.tensor_scalar_add(
                            KT[mt][:, nt * 512 : (nt + 1) * 512],
                            ps,
                            bqk_sb[:, D_T + mt : D_T + mt + 1],
                        )

            # ---------------- P2c: V_aug resident ---------------
            # Attention pools open BEFORE V emission so their SBUF/PSUM zones
            # don't land on freed P2 space (false wait-for-completion deps).
            p3tri_cm = tc.tile_pool(name="p3tri", bufs=1)
            p3tri = p3tri_cm.__enter__()
            ptp_cm = tc.tile_pool(name="ptp", bufs=4)
            ptp = ptp_cm.__enter__()
            p3s_cm = tc.tile_pool(name="p3s", bufs=2)
            p3s = p3s_cm.__enter__()
            stps_cm = tc.tile_pool(name="stps", bufs=2, space="PSUM")
            stps = stps_cm.__enter__()
            yps_cm = tc.tile_pool(name="yps", bufs=2, space="PSUM")
            yps = yps_cm.__enter__()
            tri_sb = p3tri.tile([P, 4, 2 * NQ], BF16)
            nc.sync.dma_start(
                tri_sb, trimask.rearrange("p (a q) -> p a q", q=2 * NQ)
            )

            wva_cm = tc.tile_pool(name="wvap", bufs=1)
            wva_pool = wva_cm.__enter__()
            WVA = wva_pool.tile([P, D_T, VA], BF16, name="wva")
            nc.sync.dma_start(WVA, wva.rearrange("(a p) c -> p a c", p=P))
            vchunks = [(0, 512), (512, 512), (1024, VA - 1024)]
            with tc.tile_pool(name="p2vps", bufs=2, space="PSUM", side="right") as p2vps:
                for mt in range(KT_T):
                    for ci, (c0, cw) in enumerate(vchunks):
                        ps = p2vps.tile([P, 512], F32, tag="ps")
                        for kt_ in range(D_T):
                            nc.tensor.matmul(
                                ps[:, :cw],
                                xnT[kt_][:, mt * P : (mt + 1) * P],
                                WVA[:, kt_, c0 : c0 + cw],
                                start=(kt_ == 0),
                                stop=(kt_ == D_T - 1 and not bv_nonzero),
                            )
                        if bv_nonzero:
                            nc.tensor.matmul(
                                ps[:, :cw],
                                ones1,
                                bva_sb[:, c0 : c0 + cw],
                                start=False,
                                stop=True,
                            )
                        if ci % 2 == 0:
                            nc.vector.tensor_copy(VSB[mt][:, c0 : c0 + cw], ps[:, :cw])
                        else:
                            nc.scalar.copy(VSB[mt][:, c0 : c0 + cw], ps[:, :cw])
                    if not bv_nonzero:
                        ones_cols = VSB[mt].rearrange("p (h c) -> p h c", c=HDA)[
                            :, :, HD : HD + 1
                        ]
                        nc.vector.memset(ones_cols, 1.0)

            # xnT fully consumed (Q/K/V produced) -> free it, then prefetch all
            # of wfc (bf16, pre-tiled) into the freed space during attention.
            wva_cm.__exit__(None, None, None)
            xnt_cm.__exit__(None, None, None)
            wfc_cm = tc.tile_pool(name="wfcp_sb", bufs=1, side="right")
            wfc_pool = wfc_cm.__enter__()
            WFC = wfc_pool.tile([P, M_T, D_T, P], BF16, name="WFC")
            nc.sync.dma_start(WFC, wfcp.rearrange("(a p) c -> p a c", p=P))

            # ---------------- P3: key-tile-outer attention, 8 rounds x 2 heads
            for hp in range(H // 2):
                yp = [
                    yps.tile([HDA, NQ], F32, name=f"yp{hp}_{s}", tag="yp")
                    for s in range(2)
                ]
                for kt in range(KT_T):
                    st2 = stps.tile([P, 2 * NQ], F32, tag="st2")
                    for s in range(2):
                        nc.tensor.matmul(
                            st2[:, s * NQ : (s + 1) * NQ],
                            KT[hp][s * HD : (s + 1) * HD, kt * P : (kt + 1) * P],
                            QT[hp][s * HD : (s + 1) * HD, :],
                            start=True,
                            stop=True,
                            tile_position=(s * HD, 0),
                        )
                    pt2 = ptp.tile([P, 2 * NQ], BF16, tag="pt2")
                    nc.scalar.activation(
                        pt2, st2, AF.Exp, bias=biask_sb[:, kt : kt + 1]
                    )
                    if kt >= KT_T - 4:
                        nc.vector.tensor_mul(
                            pt2, pt2, tri_sb[:, kt - (KT_T - 4), :]
                        )
                    for s in range(2):
                        h = 2 * hp + s
                        nc.tensor.matmul(
                            yp[s],
                            VSB[kt][:, h * HDA : (h + 1) * HDA],
                            pt2[:, s * NQ : (s + 1) * NQ],
                            start=(kt == 0),
                            stop=(kt == KT_T - 1),
                        )
                for s in range(2):
                    ysb = p3s.tile([HDA, NQ], F32, name=f"ysb{hp}_{s}", tag="ysb")
                    if s == 0:
                        nc.vector.tensor_copy(ysb, yp[s])
                    else:
                        nc.scalar.copy(ysb, yp[s])
                    recip = p3s.tile([1, NQ], F32, tag="recip")
                    nc.vector.reciprocal(recip, ysb[HD : HD + 1, :])
                    rb = p3s.tile([HD, NQ], F32, tag="rb")
                    nc.gpsimd.partition_broadcast(rb, recip)
                    nc.vector.tensor_mul(
                        YT[hp][s * HD : (s + 1) * HD, :], ysb[:HD, :], rb
                    )

            yps_cm.__exit__(None, None, None)
            stps_cm.__exit__(None, None, None)
            p3s_cm.__exit__(None, None, None)
            ptp_cm.__exit__(None, None, None)
            p3tri_cm.__exit__(None, None, None)
            vsb_cm.__exit__(None, None, None)
            kt_cm.__exit__(None, None, None)
            qt_cm.__exit__(None, None, None)

            # ---------------- P4: residual + LN2 + xbar-transpose ------------
            with tc.tile_pool(name="p4w", bufs=3) as p4w, tc.tile_pool(
                name="p4s", bufs=4
            ) as p4s, tc.tile_pool(name="p4ps", bufs=4, space="PSUM") as p4ps:
                xl = p4w.tile([P, Q_T, D], F32, tag="xl")
                nc.sync.dma_start(
                    xl,
                    xc[CTX - NQ :, :].rearrange("(a p) c -> p a c", p=P),
                )
                for tt in range(Q_T):
                    for mt in range(D_T):
                        tp = p4ps.tile([P, P], F32, tag="tp")
                        nc.tensor.transpose(
                            tp, YT[mt][:, tt * P : (tt + 1) * P], identity
                        )
                        nc.vector.tensor_add(
                            X2[tt][:, mt * P : (mt + 1) * P],
                            xl[:, tt, mt * P : (mt + 1) * P],
                            tp,
                        )
                    stats = p4s.tile([P, 2, 6], F32, tag="stats2")
                    for g in range(2):
                        nc.vector.bn_stats(
                            stats[:, g, :], X2[tt][:, g * 512 : (g + 1) * 512]
                        )
                    mv = p4s.tile([P, 2], F32, tag="mv2")
                    nc.vector.bn_aggr(mv, stats)
                    sd = p4s.tile([P, 1], F32, tag="sd2")
                    nc.scalar.activation(sd, mv[:, 1:2], AF.Sqrt, bias=eps_t)
                    rstd = p4s.tile([P, 1], F32, tag="rstd2")
                    nc.vector.reciprocal(rstd, sd)
                    nmb = p4s.tile([P, 1], F32, tag="nmb2")
                    nc.vector.tensor_scalar(
                        nmb, mv[:, 0:1], rstd, -1.0, ALU.mult, ALU.mult
                    )
                    l2 = p4w.tile([P, D], BF16, tag="l2")
                    nc.scalar.activation(l2, X2[tt], AF.Identity, bias=nmb, scale=rstd)
                    for mt in range(D_T):
                        tp = p4ps.tile([P, P], BF16, tag="tpb")
                        nc.tensor.transpose(
                            tp, l2[:, mt * P : (mt + 1) * P], identity_bf
                        )
                        if mt % 2 == 0:
                            nc.vector.tensor_copy(
                                L2T[mt][:, tt * P : (tt + 1) * P], tp
                            )
                        else:
                            nc.scalar.copy(L2T[mt][:, tt * P : (tt + 1) * P], tp)

            # ---------------- P5: MLP + final residual ----------------
            with tc.tile_pool(name="h1t", bufs=1) as h1t_pool, tc.tile_pool(
                name="p5w", bufs=2
            ) as p5w, tc.tile_pool(name="p5o", bufs=1) as p5o, tc.tile_pool(
                name="p5ps", bufs=3, space="PSUM"
            ) as p5ps, tc.tile_pool(
                name="p5tps", bufs=4, space="PSUM"
            ) as p5tps:
                bfc_sb = p5o.tile([P, M_T], F32)
                nc.sync.dma_start(bfc_sb, bfc[:, :])
                bproj_sb = p5o.tile([P, D_T], F32)
                nc.sync.dma_start(bproj_sb, bproj[:, :])
                OUT = p5o.tile([P, Q_T, D], F32, name="OUT")
                H1T = [h1t_pool.tile([P, NQ], BF16, name=f"H1T{i}") for i in range(M_T)]
                for mt in range(M_T):
                    ps = p5ps.tile([P, NQ], F32, tag="ps")
                    for kt_ in range(D_T):
                        nc.tensor.matmul(
                            ps,
                            WFC[:, mt, kt_, :],
                            L2T[kt_],
                            start=(kt_ == 0),
                            stop=(kt_ == D_T - 1),
                        )
                    nc.vector.tensor_scalar(
                        H1T[mt], ps, bfc_sb[:, mt : mt + 1], 0.0, ALU.add, ALU.max
                    )
                wfc_cm.__exit__(None, None, None)
                for mt in range(D_T):
                    ws = p5w.tile([P, M_T, P], BF16, tag="wsp")
                    nc.sync.dma_start(ws, wprojp[mt * P : (mt + 1) * P, :])
                    ps = p5ps.tile([P, NQ], F32, tag="ps")
                    for kt_ in range(M_T):
                        nc.tensor.matmul(
                            ps,
                            ws[:, kt_, :],
                            H1T[kt_],
                            start=(kt_ == 0),
                            stop=(kt_ == M_T - 1),
                        )
                    mlpt = p5w.tile([P, NQ], F32, tag="mlpt")
                    nc.vector.tensor_scalar_add(mlpt, ps, bproj_sb[:, mt : mt + 1])
                    for tt in range(Q_T):
                        tp = p5tps.tile([P, P], F32, tag="tp")
                        nc.tensor.transpose(
                            tp, mlpt[:, tt * P : (tt + 1) * P], identity
                        )
                        nc.vector.tensor_add(
                            OUT[:, tt, mt * P : (mt + 1) * P],
                            X2[tt][:, mt * P : (mt + 1) * P],
                            tp,
                        )
                nc.sync.dma_start(
                    out.rearrange("(a p) c -> p a c", p=P), OUT
                )

    nc.finalize()
    return nc


_PROG = {}


def _get_program(bv_nonzero: bool = False):
    if bv_nonzero not in _PROG:
        _PROG[bv_nonzero] = build_program(bv_nonzero=bv_nonzero)
    return _PROG[bv_nonzero]


def _pretile(w, n_out_tiles, n_k_tiles):
    """[K, N] -> [(n_out p), (k_tiles c)] pre-tiled lhsT layout: row
    (mt*128+p), flat col (kt*128+c) holds w[kt*128+p ... wait: value
    = w[kt*128 + p, mt*128 + c]."""
    K, N = w.shape
    assert K == n_k_tiles * P and N == n_out_tiles * P
    # axes (kt, p, mt, c) -> (mt, p, kt, c)
    return np.ascontiguousarray(
        w.reshape(n_k_tiles, P, n_out_tiles, P)
        .transpose(2, 1, 0, 3)
        .reshape(n_out_tiles * P, n_k_tiles * P)
    )


def make_in_maps(x, ln1_scale, ln1_shift, w_qkv, b_qkv, ln2_scale, ln2_shift,
                 w_fc, b_fc, w_proj, b_proj):
    """Host-side prep: fold LN affine into weights, prescale Q by 1/sqrt(hd),
    augment V with an all-ones output column per head, pre-tile all weights
    for contiguous-run DMAs, build per-core rotated context + causal bias/mask
    data."""
    import ml_dtypes

    bf16 = ml_dtypes.bfloat16

    x = np.asarray(x, np.float32)
    ln1_scale = np.asarray(ln1_scale, np.float32)
    ln1_shift = np.asarray(ln1_shift, np.float32)
    w_qkv = np.asarray(w_qkv, np.float32)
    b_qkv = np.asarray(b_qkv, np.float32)
    ln2_scale = np.asarray(ln2_scale, np.float32)
    ln2_shift = np.asarray(ln2_shift, np.float32)
    w_fc = np.asarray(w_fc, np.float32)
    b_fc = np.asarray(b_fc, np.float32)
    w_proj = np.asarray(w_proj, np.float32)
    b_proj = np.asarray(b_proj, np.float32)

    # fold LN1 affine into qkv weights
    w1 = ln1_scale[:, None] * w_qkv  # [D, 3D]
    b1 = b_qkv + ln1_shift @ w_qkv  # [3D]
    sc = 1.0 / np.sqrt(HD)
    wq = w1[:, :D] * sc
    bq = b1[:D] * sc
    wk = w1[:, D : 2 * D]
    bk = b1[D : 2 * D]
    wv = w1[:, 2 * D :]
    bv = b1[2 * D :]

    wqp_h = _pretile(wq, D_T, D_T).astype(bf16)
    wkp_h = _pretile(wk, D_T, D_T).astype(bf16)
    bqk_h = np.ascontiguousarray(
        np.concatenate([bq, bk]).reshape(2 * D_T, P).T
    )  # [128, 16] f32

    wva_h = np.zeros((D, VA), np.float32)
    bva_h = np.zeros((1, VA), np.float32)
    for h in range(H):
        wva_h[:, h * HDA : h * HDA + HD] = wv[:, h * HD : (h + 1) * HD]
        bva_h[0, h * HDA : h * HDA + HD] = bv[h * HD : (h + 1) * HD]
        bva_h[0, h * HDA + HD] = 1.0  # denominator ones column
    wva_h = wva_h.astype(bf16)
    bva_h = bva_h.astype(bf16)

    # fold LN2 affine into fc; pre-tile bf16
    wfc_f = ln2_scale[:, None] * w_fc
    wfcp_h = _pretile(wfc_f, M_T, D_T).astype(bf16)  # [4096, 1024]
    wprojp_h = _pretile(w_proj, D_T, M_T).astype(bf16)  # [1024, 4096]
    bfc_h = np.ascontiguousarray((b_fc + ln2_shift @ w_fc).reshape(M_T, P).T)
    bproj_h = np.ascontiguousarray(b_proj.reshape(D_T, P).T)  # [128, 8]

    # triangular mask for the diagonal (last) key block, S^T orientation,
    # duplicated for the head-pair layout: [128, 4, 2*NQ]
    kk = np.arange(NQ)[:, None]
    qq = np.arange(NQ)[None, :]
    tri = (kk <= qq).astype(np.float32)  # [512, 512]
    tri4 = tri.reshape(4, P, NQ)
    tri_h = np.ascontiguousarray(
        np.concatenate([tri4, tri4], axis=2).transpose(1, 0, 2).reshape(P, 4 * 2 * NQ)
    ).astype(bf16)

    in_maps = []
    for c in range(N_CORES):
        b, j = divmod(c, 4)
        xb = x[b]  # [T, D]
        xperm = np.roll(xb, -((j + 1) * NQ), axis=0)  # own chunk last
        # after roll, position block p (of 4) holds chunk (j+1+p) % 4
        bias = np.zeros(CTX, np.float32)
        for pblk in range(3):
            cp = (j + 1 + pblk) % 4
            if cp > j:
                bias[pblk * NQ : (pblk + 1) * NQ] = NEG
        biask_h = np.ascontiguousarray(bias.reshape(KT_T, P).T)  # [128, 16]
        in_maps.append(
            {
                "xc": np.ascontiguousarray(xperm),
                "xcb": np.ascontiguousarray(xperm.astype(bf16)),
                "wqp": wqp_h,
                "wkp": wkp_h,
                "bqk": bqk_h,
                "wva": wva_h,
                "bva": bva_h,
                "biask": biask_h,
                "trimask": tri_h,
                "wfcp": wfcp_h,
                "bfc": bfc_h,
                "wprojp": wprojp_h,
                "bproj": bproj_h,
            }
        )
    return in_maps


def assemble_output(results):
    out = np.empty((B, T, D), np.float32)
    for c in range(N_CORES):
        b, j = divmod(c, 4)
        out[b, j * NQ : (j + 1) * NQ, :] = results[c]["out"]
    return out


def kernel(**inputs) -> np.ndarray:
    from concourse.bass_utils import run_bass_kernel_spmd

    in_maps = make_in_maps(**inputs)
    bva = np.asarray(in_maps[0]["bva"], np.float32)[0]
    mask = np.ones(VA, bool)
    mask[HD::HDA] = False  # the ones columns
    nc = _get_program(bv_nonzero=bool(np.any(bva[mask] != 0.0)))
    res = run_bass_kernel_spmd(nc, in_maps, core_ids=list(range(N_CORES)))
    return assemble_output(res.results)



# revision 2
# speedup vs baseline: 1.5062x; 1.5062x over previous
"""Trainium2 Bass kernel for a dense transformer block (LN->causal attn->res->LN->MLP->res).

Shapes (hardcoded): x [2, 2048, 1024], 16 heads, head_dim 64, MLP hidden 4096, fp32 out.

Sharding: 8 cores = (batch b in {0,1}) x (sequence chunk j in {0..3}, 512 tokens).
Each core receives its batch's full 2048-token context, ROTATED so that its own
chunk sits in the last 512 positions.  Causality is enforced by per-core DATA:
a per-key-tile additive bias (0 past / -30000 future) folded into the exp on
ACT, plus a triangular 0/1 mask multiplied onto the diagonal (last 4) key
tiles.  Each core computes LN1 + K/V over the whole context, Q/attention/LN2/
MLP for its own 512 tokens, and writes its [512, 1024] output slice.  No
cross-core communication.

v3 notes (vs v2):
  * ALL inputs packed into ONE uint8 DRAM tensor (the per-exec runtime staging
    cost is ~63us per input tensor + ~10-16us/MB; 13 tensors -> 1).
  * Weights shipped as fp8e4m3 scaled x256 (halves input bytes); used DIRECTLY
    as the stationary matmul operand against bf16 moving (mixed dtypes are
    legal), with the 1/256 folded into each epilogue's scale.
  * x shipped bf16-only (residual base is bf16-rounded; adds ~2.7e-3 max-rel
    which fits the 2e-2 budget).  Own-chunk x kept SBUF-resident from P1 for
    the P4 residual (no reload).
  * trimask shipped fp8 (0/1) and widened to bf16 on-core once.
"""

from contextlib import ExitStack

import numpy as np

import concourse.bacc as bacc
import concourse.mybir as mybir
import concourse.tile as tile
from concourse.masks import make_identity

F32 = mybir.dt.float32
BF16 = mybir.dt.bfloat16
FP8 = mybir.dt.float8e4
AF = mybir.ActivationFunctionType
ALU = mybir.AluOpType

B = 2
T = 2048
D = 1024
H = 16
HD = 64
HDA = HD + 1  # +1 denominator column per head
MLP = 4096
NQ = 512  # tokens per core
CTX = T
EPS = 1e-5
NEG = -30000.0

N_CORES = 8
P = 128

KT_T = CTX // P  # 16 key tiles
D_T = D // P  # 8
Q_T = NQ // P  # 4
M_T = MLP // P  # 32
VA = H * HDA  # 1040 augmented V width

WS = 256.0  # fp8 weight scale
WSI = 1.0 / WS

# ---- packed-input layout (bytes). All segments 4KB-aligned. ----
def _align(x, a=4096):
    return (x + a - 1) // a * a


_off = 0
def _seg(nbytes):
    global _off
    o = _off
    _off = _align(_off + nbytes)
    return o


OFF_X = _seg(CTX * D * 2)           # bf16 [2048,1024] rotated
OFF_WQ = _seg(D * D)                # fp8 [1024,1024] pretiled (x256)
OFF_WK = _seg(D * D)                # fp8 [1024,1024] pretiled (x256)
OFF_WVA = _seg(D * VA)              # fp8 [1024,1040] (x256)
OFF_WFC = _seg(MLP * D)             # fp8 [4096,1024] pretiled (x256)
OFF_WPJ = _seg(D * MLP)             # fp8 [1024,4096] pretiled (x256)
OFF_TRI = _seg(P * 4 * 2 * NQ)      # fp8 0/1 [128, 4096]
OFF_BQK = _seg(P * 2 * D_T * 4)     # f32 [128,16]
OFF_BIASK = _seg(P * KT_T * 4)      # f32 [128,16]
OFF_BFC = _seg(P * M_T * 4)         # f32 [128,32]
OFF_BPJ = _seg(P * D_T * 4)         # f32 [128,8]
OFF_BVA = _seg(VA * 2)              # bf16 [1,1040] (x256)
NB = _align(_off)


def build_program(loop_n: int = 1, bv_nonzero: bool = False):
    """Emit the SPMD Bass program. Returns finalized nc."""
    nc = bacc.Bacc("TRN2", target_bir_lowering=False)

    pk = nc.dram_tensor("pk", [1, NB], mybir.dt.uint8, kind="ExternalInput")
    out = nc.dram_tensor("out", [NQ, D], F32, kind="ExternalOutput")

    def view(off, nbytes, dt):
        return pk[0, off : off + nbytes].bitcast(dt)

    xcb_v = view(OFF_X, CTX * D * 2, BF16)  # flat (CTX*D,)

    with tile.TileContext(nc) as tc:
        with ExitStack() as ctx:
            if loop_n > 1:
                ctx.enter_context(tc.For_i(0, loop_n, 1))
            const = ctx.enter_context(tc.tile_pool(name="const", bufs=1))
            identity = const.tile([P, P], F32)
            make_identity(nc, identity)
            identity_bf = const.tile([P, P], BF16)
            make_identity(nc, identity_bf)
            ones1 = const.tile([1, P], BF16)
            nc.vector.memset(ones1, 1.0)
            eps_t = const.tile([P, 1], F32)
            nc.vector.memset(eps_t, EPS)
            bqk_sb = const.tile([P, 2 * D_T], F32)
            nc.sync.dma_start(
                bqk_sb, view(OFF_BQK, P * 2 * D_T * 4, F32).rearrange("(p c) -> p c", p=P)
            )
            bva_sb = const.tile([1, VA], BF16)
            nc.sync.dma_start(
                bva_sb, view(OFF_BVA, VA * 2, BF16).rearrange("(p c) -> p c", p=1)
            )
            biask_sb = const.tile([P, KT_T], F32)
            nc.sync.dma_start(
                biask_sb, view(OFF_BIASK, P * KT_T * 4, F32).rearrange("(p c) -> p c", p=P)
            )

            # Long-lived pools (closed explicitly at phase boundaries).
            qt_cm = tc.tile_pool(name="qt", bufs=1)
            qt_pool = qt_cm.__enter__()
            QT = [qt_pool.tile([P, NQ], BF16, name=f"QT{i}") for i in range(D_T)]
            kt_cm = tc.tile_pool(name="ktp", bufs=1)
            kt_pool = kt_cm.__enter__()
            KT = [kt_pool.tile([P, CTX], BF16, name=f"KT{i}") for i in range(D_T)]
            vsb_cm = tc.tile_pool(name="vsb", bufs=1)
            vsb_pool = vsb_cm.__enter__()
            VSB = [vsb_pool.tile([P, VA], BF16, name=f"VSB{i}") for i in range(KT_T)]

            # RIGHT pools (live into P4/P5)
            yt_pool = ctx.enter_context(tc.tile_pool(name="yt", bufs=1, side="right"))
            YT = [yt_pool.tile([P, NQ], F32, name=f"YT{i}") for i in range(D_T)]
            x2_pool = ctx.enter_context(tc.tile_pool(name="x2", bufs=1, side="right"))
            X2 = [x2_pool.tile([P, D], F32, name=f"X2{i}") for i in range(Q_T)]
            l2t_pool = ctx.enter_context(
                tc.tile_pool(name="l2t", bufs=1, side="right")
            )
            L2T = [l2t_pool.tile([P, NQ], BF16, name=f"L2T{i}") for i in range(D_T)]
            # own-chunk x (bf16) stays resident for the P4 residual
            xo_pool = ctx.enter_context(tc.tile_pool(name="xo", bufs=1, side="right"))
            XO = xo_pool.tile([P, 4, D], BF16, name="XO")
            # xnt sits on TOP of the RIGHT stack so it can be closed right
            # after V production; wfc prefetch then reuses its space.
            xnt_cm = tc.tile_pool(name="xnt", bufs=1, side="right")
            xnt_pool = xnt_cm.__enter__()
            xnT = [xnt_pool.tile([P, CTX], BF16, name=f"xnT{i}") for i in range(D_T)]

            # ---------------- P1: load x, LN1, PE-transpose -> xnT (bf16) ----
            # Own-chunk tiles (tt 12..15) first so Q production can start
            # early; x loaded in 4-tile batches (one DMA each).
            with tc.tile_pool(name="p1work", bufs=2) as p1w, tc.tile_pool(
                name="p1xn", bufs=4
            ) as p1xn, tc.tile_pool(name="p1stat", bufs=6) as p1s, tc.tile_pool(
                name="p1ps", bufs=6, space="PSUM"
            ) as p1ps:
                for bt in ((12, 13, 14, 15), (0, 1, 2, 3), (4, 5, 6, 7), (8, 9, 10, 11)):
                    if bt[0] == 12:
                        xt = XO
                    else:
                        xt = p1w.tile([P, 4, D], BF16, tag="xt")
                    nc.sync.dma_start(
                        xt,
                        xcb_v[bt[0] * P * D : (bt[0] + 4) * P * D].rearrange(
                            "(a p c) -> p a c", p=P, c=D
                        ),
                    )
                    xns = []
                    for ai, tt in enumerate(bt):
                        stats = p1s.tile([P, 2, 6], F32, tag="stats")
                        for g in range(2):
                            nc.vector.bn_stats(
                                stats[:, g, :], xt[:, ai, g * 512 : (g + 1) * 512]
                            )
                        mv = p1s.tile([P, 2], F32, tag="mv")
                        nc.vector.bn_aggr(mv, stats)
                        sd = p1s.tile([P, 1], F32, tag="sd")
                        nc.scalar.activation(sd, mv[:, 1:2], AF.Sqrt, bias=eps_t)
                        rstd = p1s.tile([P, 1], F32, tag="rstd")
                        nc.vector.reciprocal(rstd, sd)
                        nmb = p1s.tile([P, 1], F32, tag="nmb")
                        nc.vector.tensor_scalar(
                            nmb, mv[:, 0:1], rstd, -1.0, ALU.mult, ALU.mult
                        )
                        xn = p1xn.tile([P, D], BF16, tag="xn")
                        nc.scalar.activation(
                            xn, xt[:, ai, :], AF.Identity, bias=nmb, scale=rstd
                        )
                        xns.append(xn)
                    # 4 transposes per dt into one PSUM tile, then a single
                    # 512-wide copy.
                    for dt_ in range(D_T):
                        tp = p1ps.tile([P, 4, P], BF16, tag="tp")
                        for ai in range(4):
                            nc.tensor.transpose(
                                tp[:, ai, :],
                                xns[ai][:, dt_ * P : (dt_ + 1) * P],
                                identity_bf,
                            )
                        if dt_ % 2 == 0:
                            nc.vector.tensor_copy(
                                xnT[dt_][:, bt[0] * P : (bt[0] + 4) * P], tp
                            )
                        else:
                            nc.scalar.copy(
                                xnT[dt_][:, bt[0] * P : (bt[0] + 4) * P], tp
                            )

            # ---------------- P2a: Q^T (bf16, own 512 tokens) ----------------
            with tc.tile_pool(name="p2q", bufs=2) as p2q, tc.tile_pool(
                name="p2qps", bufs=3, space="PSUM"
            ) as p2qps:
                for mt in range(D_T):
                    ws = p2q.tile([P, D_T, P], FP8, tag="wsq")
                    nc.sync.dma_start(
                        ws,
                        view(OFF_WQ + mt * P * D, P * D, FP8).rearrange(
                            "(p a c) -> p a c", p=P, c=P
                        ),
                    )
                    ps = p2qps.tile([P, NQ], F32, tag="ps")
                    for kt_ in range(D_T):
                        nc.tensor.matmul(
                            ps,
                            ws[:, kt_, :],
                            xnT[kt_][:, CTX - NQ :],
                            start=(kt_ == 0),
                            stop=(kt_ == D_T - 1),
                        )
                    nc.scalar.activation(
                        QT[mt], ps, AF.Identity, bias=bqk_sb[:, mt : mt + 1], scale=WSI
                    )

            # ---------------- P2b: K^T (bf16, SBUF resident, full ctx) -------
            with tc.tile_pool(name="p2k", bufs=2) as p2k, tc.tile_pool(
                name="p2kps", bufs=3, space="PSUM"
            ) as p2kps:
                for mt in range(D_T):
                    ws = p2k.tile([P, D_T, P], FP8, tag="wsk")
                    nc.sync.dma_start(
                        ws,
                        view(OFF_WK + mt * P * D, P * D, FP8).rearrange(
                            "(p a c) -> p a c", p=P, c=P
                        ),
                    )
                    for nt in range(CTX // 512):
                        ps = p2kps.tile([P, 512], F32, tag="ps")
                        for kt_ in range(D_T):
                            nc.tensor.matmul(
                                ps,
                                ws[:, kt_, :],
                                xnT[kt_][:, nt * 512 : (nt + 1) * 512],
                                start=(kt_ == 0),
                                stop=(kt_ == D_T - 1),
                            )
                        nc.vector.tensor_scalar(
                            KT[mt][:, nt * 512 : (nt + 1) * 512],
                            ps,
                            WSI,
                            bqk_sb[:, D_T + mt : D_T + mt + 1],
                            ALU.mult,
                            ALU.add,
                        )

            # ---------------- P2c: V_aug resident ---------------
            # Attention pools open BEFORE V emission so their SBUF/PSUM zones
            # don't land on freed P2 space (false wait-for-completion deps).
            p3tri_cm = tc.tile_pool(name="p3tri", bufs=1)
            p3tri = p3tri_cm.__enter__()
            ptp_cm = tc.tile_pool(name="ptp", bufs=4)
            ptp = ptp_cm.__enter__()
            p3s_cm = tc.tile_pool(name="p3s", bufs=2)
            p3s = p3s_cm.__enter__()
            stps_cm = tc.tile_pool(name="stps", bufs=2, space="PSUM")
            stps = stps_cm.__enter__()
            yps_cm = tc.tile_pool(name="yps", bufs=2, space="PSUM")
            yps = yps_cm.__enter__()
            tri8 = p3tri.tile([P, 4, 2 * NQ], FP8)
            nc.sync.dma_start(
                tri8,
                view(OFF_TRI, P * 4 * 2 * NQ, FP8).rearrange(
                    "(p a q) -> p a q", p=P, q=2 * NQ
                ),
            )
            tri_sb = p3tri.tile([P, 4, 2 * NQ], BF16)
            nc.vector.tensor_copy(tri_sb, tri8)

            wva_cm = tc.tile_pool(name="wvap", bufs=1)
            wva_pool = wva_cm.__enter__()
            WVA = wva_pool.tile([P, D_T, VA], FP8, name="wva")
            nc.sync.dma_start(
                WVA,
                view(OFF_WVA, D * VA, FP8).rearrange("(a p c) -> p a c", p=P, c=VA),
            )
            vchunks = [(0, 512), (512, 512), (1024, VA - 1024)]
            with tc.tile_pool(name="p2vps", bufs=2, space="PSUM", side="right") as p2vps:
                for mt in range(KT_T):
                    for ci, (c0, cw) in enumerate(vchunks):
                        ps = p2vps.tile([P, 512], F32, tag="ps")
                        for kt_ in range(D_T):
                            nc.tensor.matmul(
                                ps[:, :cw],
                                xnT[kt_][:, mt * P : (mt + 1) * P],
                                WVA[:, kt_, c0 : c0 + cw],
                                start=(kt_ == 0),
                                stop=(kt_ == D_T - 1 and not bv_nonzero),
                            )
                        if bv_nonzero:
                            nc.tensor.matmul(
                                ps[:, :cw],
                                ones1,
                                bva_sb[:, c0 : c0 + cw],
                                start=False,
                                stop=True,
                            )
                        if ci % 2 == 0:
                            nc.vector.tensor_scalar_mul(
                                VSB[mt][:, c0 : c0 + cw], ps[:, :cw], WSI
                            )
                        else:
                            nc.scalar.activation(
                                VSB[mt][:, c0 : c0 + cw],
                                ps[:, :cw],
                                AF.Identity,
                                scale=WSI,
                            )
                    if not bv_nonzero:
                        ones_cols = VSB[mt].rearrange("p (h c) -> p h c", c=HDA)[
                            :, :, HD : HD + 1
                        ]
                        nc.vector.memset(ones_cols, 1.0)

            # xnT fully consumed (Q/K/V produced) -> free it, then prefetch all
            # of wfc (fp8, pre-tiled) into the freed space during attention.
            wva_cm.__exit__(None, None, None)
            xnt_cm.__exit__(None, None, None)
            wfc_cm = tc.tile_pool(name="wfcp_sb", bufs=1, side="right")
            wfc_pool = wfc_cm.__enter__()
            WFC = wfc_pool.tile([P, M_T, D_T, P], FP8, name="WFC")
            nc.sync.dma_start(
                WFC,
                view(OFF_WFC, MLP * D, FP8).rearrange(
                    "(a p c) -> p a c", p=P, c=D
                ).rearrange("p a (k c) -> p a k c", c=P),
            )

            # ---------------- P3: key-tile-outer attention, 8 rounds x 2 heads
            for hp in range(H // 2):
                yp = [
                    yps.tile([HDA, NQ], F32, name=f"yp{hp}_{s}", tag="yp")
                    for s in range(2)
                ]
                for kt in range(KT_T):
                    st2 = stps.tile([P, 2 * NQ], F32, tag="st2")
                    for s in range(2):
                        nc.tensor.matmul(
                            st2[:, s * NQ : (s + 1) * NQ],
                            KT[hp][s * HD : (s + 1) * HD, kt * P : (kt + 1) * P],
                            QT[hp][s * HD : (s + 1) * HD, :],
                            start=True,
                            stop=True,
                            tile_position=(s * HD, 0),
                        )
                    pt2 = ptp.tile([P, 2 * NQ], BF16, tag="pt2")
                    nc.scalar.activation(
                        pt2, st2, AF.Exp, bias=biask_sb[:, kt : kt + 1]
                    )
                    if kt >= KT_T - 4:
                        nc.vector.tensor_mul(
                            pt2, pt2, tri_sb[:, kt - (KT_T - 4), :]
                        )
                    for s in range(2):
                        h = 2 * hp + s
                        nc.tensor.matmul(
                            yp[s],
                            VSB[kt][:, h * HDA : (h + 1) * HDA],
                            pt2[:, s * NQ : (s + 1) * NQ],
                            start=(kt == 0),
                            stop=(kt == KT_T - 1),
                        )
                for s in range(2):
                    ysb = p3s.tile([HDA, NQ], F32, name=f"ysb{hp}_{s}", tag="ysb")
                    if s == 0:
                        nc.vector.tensor_copy(ysb, yp[s])
                    else:
                        nc.scalar.copy(ysb, yp[s])
                    recip = p3s.tile([1, NQ], F32, tag="recip")
                    nc.vector.reciprocal(recip, ysb[HD : HD + 1, :])
                    rb = p3s.tile([HD, NQ], F32, tag="rb")
                    nc.gpsimd.partition_broadcast(rb, recip)
                    nc.vector.tensor_mul(
                        YT[hp][s * HD : (s + 1) * HD, :], ysb[:HD, :], rb
                    )

            yps_cm.__exit__(None, None, None)
            stps_cm.__exit__(None, None, None)
            p3s_cm.__exit__(None, None, None)
            ptp_cm.__exit__(None, None, None)
            p3tri_cm.__exit__(None, None, None)
            vsb_cm.__exit__(None, None, None)
            kt_cm.__exit__(None, None, None)
            qt_cm.__exit__(None, None, None)

            # ---------------- P4: residual + LN2 + xbar-transpose ------------
            with tc.tile_pool(name="p4w", bufs=3) as p4w, tc.tile_pool(
                name="p4s", bufs=4
            ) as p4s, tc.tile_pool(name="p4ps", bufs=4, space="PSUM") as p4ps:
                for tt in range(Q_T):
                    for mt in range(D_T):
                        tp = p4ps.tile([P, P], F32, tag="tp")
                        nc.tensor.transpose(
                            tp, YT[mt][:, tt * P : (tt + 1) * P], identity
                        )
                        nc.vector.tensor_add(
                            X2[tt][:, mt * P : (mt + 1) * P],
                            XO[:, tt, mt * P : (mt + 1) * P],
                            tp,
                        )
                    stats = p4s.tile([P, 2, 6], F32, tag="stats2")
                    for g in range(2):
                        nc.vector.bn_stats(
                            stats[:, g, :], X2[tt][:, g * 512 : (g + 1) * 512]
                        )
                    mv = p4s.tile([P, 2], F32, tag="mv2")
                    nc.vector.bn_aggr(mv, stats)
                    sd = p4s.tile([P, 1], F32, tag="sd2")
                    nc.scalar.activation(sd, mv[:, 1:2], AF.Sqrt, bias=eps_t)
                    rstd = p4s.tile([P, 1], F32, tag="rstd2")
                    nc.vector.reciprocal(rstd, sd)
                    nmb = p4s.tile([P, 1], F32, tag="nmb2")
                    nc.vector.tensor_scalar(
                        nmb, mv[:, 0:1], rstd, -1.0, ALU.mult, ALU.mult
                    )
                    l2 = p4w.tile([P, D], BF16, tag="l2")
                    nc.scalar.activation(l2, X2[tt], AF.Identity, bias=nmb, scale=rstd)
                    for mt in range(D_T):
                        tp = p4ps.tile([P, P], BF16, tag="tpb")
                        nc.tensor.transpose(
                            tp, l2[:, mt * P : (mt + 1) * P], identity_bf
                        )
                        if mt % 2 == 0:
                            nc.vector.tensor_copy(
                                L2T[mt][:, tt * P : (tt + 1) * P], tp
                            )
                        else:
                            nc.scalar.copy(L2T[mt][:, tt * P : (tt + 1) * P], tp)

            # ---------------- P5: MLP + final residual ----------------
            with tc.tile_pool(name="h1t", bufs=1) as h1t_pool, tc.tile_pool(
                name="p5w", bufs=2
            ) as p5w, tc.tile_pool(name="p5o", bufs=1) as p5o, tc.tile_pool(
                name="p5ps", bufs=3, space="PSUM"
            ) as p5ps, tc.tile_pool(
                name="p5tps", bufs=4, space="PSUM"
            ) as p5tps:
                bfc_sb = p5o.tile([P, M_T], F32)
                nc.sync.dma_start(
                    bfc_sb,
                    view(OFF_BFC, P * M_T * 4, F32).rearrange("(p c) -> p c", p=P),
                )
                bproj_sb = p5o.tile([P, D_T], F32)
                nc.sync.dma_start(
                    bproj_sb,
                    view(OFF_BPJ, P * D_T * 4, F32).rearrange("(p c) -> p c", p=P),
                )
                OUT = p5o.tile([P, Q_T, D], F32, name="OUT")
                H1T = [h1t_pool.tile([P, NQ], BF16, name=f"H1T{i}") for i in range(M_T)]
                for mt in range(M_T):
                    ps = p5ps.tile([P, NQ], F32, tag="ps")
                    for kt_ in range(D_T):
                        nc.tensor.matmul(
                            ps,
                            WFC[:, mt, kt_, :],
                            L2T[kt_],
                            start=(kt_ == 0),
                            stop=(kt_ == D_T - 1),
                        )
                    nc.scalar.activation(
                        H1T[mt], ps, AF.Relu, bias=bfc_sb[:, mt : mt + 1], scale=WSI
                    )
                wfc_cm.__exit__(None, None, None)
                for mt in range(D_T):
                    ws = p5w.tile([P, M_T, P], FP8, tag="wsp")
                    nc.sync.dma_start(
                        ws,
                        view(OFF_WPJ + mt * P * MLP, P * MLP, FP8).rearrange(
                            "(p a c) -> p a c", p=P, c=P
                        ),
                    )
                    ps = p5ps.tile([P, NQ], F32, tag="ps")
                    for kt_ in range(M_T):
                        nc.tensor.matmul(
                            ps,
                            ws[:, kt_, :],
                            H1T[kt_],
                            start=(kt_ == 0),
                            stop=(kt_ == M_T - 1),
                        )
                    mlpt = p5w.tile([P, NQ], F32, tag="mlpt")
                    nc.vector.tensor_scalar(
                        mlpt, ps, WSI, bproj_sb[:, mt : mt + 1], ALU.mult, ALU.add
                    )
                    for tt in range(Q_T):
                        tp = p5tps.tile([P, P], F32, tag="tp")
                        nc.tensor.transpose(
                            tp, mlpt[:, tt * P : (tt + 1) * P], identity
                        )
                        nc.vector.tensor_add(
                            OUT[:, tt, mt * P : (mt + 1) * P],
                            X2[tt][:, mt * P : (mt + 1) * P],
                            tp,
                        )
                nc.sync.dma_start(
                    out.rearrange("(a p) c -> p a c", p=P), OUT
                )

    nc.finalize()
    return nc


_PROG = {}


def _get_program(bv_nonzero: bool = False):
    if bv_nonzero not in _PROG:
        _PROG[bv_nonzero] = build_program(bv_nonzero=bv_nonzero)
    return _PROG[bv_nonzero]


def _pretile(w, n_out_tiles, n_k_tiles):
    """[K, N] -> lhsT pre-tiled layout: row (mt*128+p), flat col (kt*128+c)
    holds w[kt*128 + p, mt*128 + c]."""
    K, N = w.shape
    assert K == n_k_tiles * P and N == n_out_tiles * P
    # axes (kt, p, mt, c) -> (mt, p, kt, c)
    return np.ascontiguousarray(
        w.reshape(n_k_tiles, P, n_out_tiles, P)
        .transpose(2, 1, 0, 3)
        .reshape(n_out_tiles * P, n_k_tiles * P)
    )


def make_in_maps(x, ln1_scale, ln1_shift, w_qkv, b_qkv, ln2_scale, ln2_shift,
                 w_fc, b_fc, w_proj, b_proj):
    """Host-side prep: fold LN affine into weights, prescale Q by 1/sqrt(hd),
    augment V with an all-ones output column per head, pre-tile + fp8-quantize
    all weights (x256), build per-core rotated context + causal bias/mask
    data, and pack EVERYTHING into one uint8 tensor per core."""
    import ml_dtypes

    bf16 = ml_dtypes.bfloat16
    fp8 = mybir.dt.np(FP8)

    x = np.asarray(x, np.float32)
    ln1_scale = np.asarray(ln1_scale, np.float32)
    ln1_shift = np.asarray(ln1_shift, np.float32)
    w_qkv = np.asarray(w_qkv, np.float32)
    b_qkv = np.asarray(b_qkv, np.float32)
    ln2_scale = np.asarray(ln2_scale, np.float32)
    ln2_shift = np.asarray(ln2_shift, np.float32)
    w_fc = np.asarray(w_fc, np.float32)
    b_fc = np.asarray(b_fc, np.float32)
    w_proj = np.asarray(w_proj, np.float32)
    b_proj = np.asarray(b_proj, np.float32)

    # fold LN1 affine into qkv weights
    w1 = ln1_scale[:, None] * w_qkv  # [D, 3D]
    b1 = b_qkv + ln1_shift @ w_qkv  # [3D]
    sc = 1.0 / np.sqrt(HD)
    wq = w1[:, :D] * sc
    bq = b1[:D] * sc
    wk = w1[:, D : 2 * D]
    bk = b1[D : 2 * D]
    wv = w1[:, 2 * D :]
    bv = b1[2 * D :]

    wqp_h = (_pretile(wq, D_T, D_T) * WS).astype(fp8)
    wkp_h = (_pretile(wk, D_T, D_T) * WS).astype(fp8)
    bqk_h = np.ascontiguousarray(
        np.concatenate([bq, bk]).reshape(2 * D_T, P).T
    )  # [128, 16] f32

    wva_h = np.zeros((D, VA), np.float32)
    bva_h = np.zeros((1, VA), np.float32)
    for h in range(H):
        wva_h[:, h * HDA : h * HDA + HD] = wv[:, h * HD : (h + 1) * HD]
        bva_h[0, h * HDA : h * HDA + HD] = bv[h * HD : (h + 1) * HD]
        bva_h[0, h * HDA + HD] = 1.0  # denominator ones column
    wva_h = (wva_h * WS).astype(fp8)
    bva_h = (bva_h * WS).astype(bf16)

    # fold LN2 affine into fc; pre-tile fp8
    wfc_f = ln2_scale[:, None] * w_fc
    wfcp_h = (_pretile(wfc_f, M_T, D_T) * WS).astype(fp8)  # [4096, 1024]
    wprojp_h = (_pretile(w_proj, D_T, M_T) * WS).astype(fp8)  # [1024, 4096]
    bfc_h = np.ascontiguousarray((b_fc + ln2_shift @ w_fc).reshape(M_T, P).T)
    bproj_h = np.ascontiguousarray(b_proj.reshape(D_T, P).T)  # [128, 8]

    # triangular mask for the diagonal (last) key block, S^T orientation,
    # duplicated for the head-pair layout: [128, 4, 2*NQ]
    kk = np.arange(NQ)[:, None]
    qq = np.arange(NQ)[None, :]
    tri = (kk <= qq).astype(np.float32)  # [512, 512]
    tri4 = tri.reshape(4, P, NQ)
    tri_h = np.ascontiguousarray(
        np.concatenate([tri4, tri4], axis=2).transpose(1, 0, 2).reshape(P, 4 * 2 * NQ)
    ).astype(fp8)

    def put(buf, off, arr):
        b = np.ascontiguousarray(arr).view(np.uint8).reshape(-1)
        buf[off : off + b.size] = b

    base = np.zeros(NB, np.uint8)
    put(base, OFF_WQ, wqp_h)
    put(base, OFF_WK, wkp_h)
    put(base, OFF_WVA, wva_h)
    put(base, OFF_WFC, wfcp_h)
    put(base, OFF_WPJ, wprojp_h)
    put(base, OFF_TRI, tri_h)
    put(base, OFF_BQK, bqk_h)
    put(base, OFF_BFC, bfc_h)
    put(base, OFF_BPJ, bproj_h)
    put(base, OFF_BVA, bva_h)

    in_maps = []
    for c in range(N_CORES):
        b, j = divmod(c, 4)
        xb = x[b]  # [T, D]
        xperm = np.roll(xb, -((j + 1) * NQ), axis=0)  # own chunk last
        # after roll, position block p (of 4) holds chunk (j+1+p) % 4
        bias = np.zeros(CTX, np.float32)
        for pblk in range(3):
            cp = (j + 1 + pblk) % 4
            if cp > j:
                bias[pblk * NQ : (pblk + 1) * NQ] = NEG
        biask_h = np.ascontiguousarray(bias.reshape(KT_T, P).T)  # [128, 16]
        pkc = base.copy()
        put(pkc, OFF_X, xperm.astype(bf16))
        put(pkc, OFF_BIASK, biask_h)
        in_maps.append({"pk": pkc.reshape(1, NB)})
    return in_maps


def assemble_output(results):
    out = np.empty((B, T, D), np.float32)
    for c in range(N_CORES):
        b, j = divmod(c, 4)
        out[b, j * NQ : (j + 1) * NQ, :] = results[c]["out"]
    return out


def kernel(**inputs) -> np.ndarray:
    from concourse.bass_utils import run_bass_kernel_spmd

    in_maps = make_in_maps(**inputs)
    bva = np.frombuffer(
        in_maps[0]["pk"][0, OFF_BVA : OFF_BVA + VA * 2].tobytes(),
        dtype=mybir.dt.np(BF16),
    ).astype(np.float32)
    mask = np.ones(VA, bool)
    mask[HD::HDA] = False  # the ones columns
    nc = _get_program(bv_nonzero=bool(np.any(bva[mask] != 0.0)))
    res = run_bass_kernel_spmd(nc, in_maps, core_ids=list(range(N_CORES)))
    return assemble_output(res.results)
